# revision 5
# baseline (speedup 1.0000x reference)
"""Trainium2 Bass kernel for nn_HGraphConv (4-hop masked-softmax graph conv).

Math per hop k:  out_k = softmax(where(m_k, E_k, NEG), axis=1) @ (x @ W_k)
Final:           concat(out_0..out_3, axis=2) + bias

Device strategy (data-parallel over batch B=64 across 8 cores, 8 batches/core):
  - P_k = exp(E_k) * m_k  computed in transposed [j, i] layout so 128x128
    slices are directly the matmul stationary operand (no on-chip transposes).
    Masked entries are exactly 0, matching softmax-with-NEG-fill exactly
    (no empty mask rows for this graph; checked on host with a numpy patch
    as fallback).
  - H_k[j, (b,f)] = x[b] @ W_k computed on device from x.T shards.
  - out[i, (b,f)] += P.T-tile @ H-tile accumulated over j in PSUM; the row
    sum Z[i] comes from one extra N=1 matmul vs a ones-vector that reuses
    the already-loaded stationary tile.
  - Eviction fuses the 1/Z softmax normalization (per-partition scalar) and
    the bias add (free-dim vector) in one scalar_tensor_tensor op:
        out = psum * (1/Z) + bias_rep   (valid because softmax rows sum to 1)
  - Hop 0 has m_0 = I => A_0 = I exactly, so out_0 = x @ W_0 + bias_0 and
    E_0/m_0 are never loaded (verified on host, numpy fallback otherwise).
"""

import os
import sys

import numpy as np

sys.path.insert(0, "/opt/trn_rl_repo")
sys.path.insert(0, "/opt/trn_rl_repo/concourse")

import concourse.bass as bass  # noqa: E402
import concourse.mybir as mybir  # noqa: E402
import concourse.tile as tile  # noqa: E402
import concourse.bass_utils as _bu  # noqa: E402
import concourse.bass2jax as _b2j  # noqa: E402
from concourse.bass_utils import run_bass_kernel_spmd  # noqa: E402

# ---------------------------------------------------------------------------
# Workaround for this walrus build: the TRN2 ISA has exactly one sync-wait
# slot per 64B instruction, and this compiler errors ("Too many sync wait
# commands") instead of splitting multi-wait instructions emitted by Tile.
# Split them ourselves at the BIR-JSON level: hoist all but one wait onto
# single-wait NoOps inserted right before the instruction on the same engine
# queue (queue waits execute in order, so this is semantically identical).
# ---------------------------------------------------------------------------
import json as _json  # noqa: E402


def _split_multi_waits_json(bir_json):
    if isinstance(bir_json, (bytes, bytearray)):
        m = _json.loads(bir_json.decode())
    else:
        m = _json.loads(bir_json)
    ctr = 0
    for fn in m["functions"]:
        for blk in fn["blocks"]:
            out = []
            for inst in blk["instructions"]:
                si = inst.get("sync_info")
                if si:
                    ws = si.get("on_wait") or []
                    if len(ws) > 1:
                        for w in ws[:-1]:
                            ctr += 1
                            out.append(
                                {
                                    "debug": inst.get("debug", 0),
                                    "engine": inst["engine"],
                                    "ins": [],
                                    "name": f"WX-{ctr}",
                                    "opcode": "NoOp",
                                    "outs": [],
                                    "text_hint": "split_wait",
                                    "sync_info": {
                                        "on_update": [],
                                        "on_wait": [w],
                                    },
                                }
                            )
                        si["on_wait"] = [ws[-1]]
                    us = si.get("on_update") or []
                    if len(us) > 1:
                        raise RuntimeError(
                            f"multi-update inst {inst['name']}: unsupported"
                        )
                out.append(inst)
            blk["instructions"] = out
    return _json.dumps(m).encode()


_orig_compile_bir_kernel = _bu.compile_bir_kernel.__wrapped__ if hasattr(
    _bu.compile_bir_kernel, "__wrapped__"
) else _bu.compile_bir_kernel


def _patched_compile_bir_kernel(bir_json, tmpdir, neff_name="file.neff"):
    return _orig_compile_bir_kernel(
        _split_multi_waits_json(bir_json), tmpdir, neff_name
    )


_bu.compile_bir_kernel = _patched_compile_bir_kernel
if hasattr(_b2j, "compile_bir_kernel"):
    _b2j.compile_bir_kernel = _patched_compile_bir_kernel

N_CORES = 8
B = 64
N = 1024
F = 128
HOPS = 4
NEG = -9.0e15

# filled by kernel() for test.py to read
last_run_info = {}


def build_nc(b_local: int, n: int, f: int = 128):
    """Build the per-core Bass module.

    b_local: batches per core.  n: graph nodes.  f: feature dim (=128).
    Requires b_local*f either <=512 or a multiple of 512, n % 128 == 0.
    """
    P = 128
    assert f == 128 and n % P == 0
    hc = b_local * f            # H columns per j-chunk
    assert hc <= 512 or hc % 512 == 0
    n_half = max(1, hc // 512)  # matmul column splits of H
    hw = min(hc, 512)           # moving-operand width per matmul
    bg = hw // f                # batches per column split (<=4)
    nch = n // P                # number of 128-row chunks (j and i)
    khops = HOPS - 1            # hops that need attention (1..3)

    nc = bass.Bass()
    fp32 = mybir.dt.float32
    xt_d = nc.dram_tensor("xt", [b_local, f, n], fp32, kind="ExternalInput")
    et_d = nc.dram_tensor("et", [khops, n, n], fp32, kind="ExternalInput")
    mt_d = nc.dram_tensor("mt", [khops, n, n], mybir.dt.uint8, kind="ExternalInput")
    wc_d = nc.dram_tensor("wc", [f, HOPS * f], fp32, kind="ExternalInput")
    bias_d = nc.dram_tensor("bias", [HOPS * f], fp32, kind="ExternalInput")
    out_d = nc.dram_tensor("out", [HOPS, b_local, n, f], fp32, kind="ExternalOutput")

    with tile.TileContext(nc) as tc:
        with (
            tc.tile_pool(name="const", bufs=1) as const,
            tc.tile_pool(name="pt", bufs=2) as ptp,
            tc.tile_pool(name="mk", bufs=1) as mkp,
            tc.tile_pool(name="hh", bufs=2) as hhp,
            tc.tile_pool(name="stage", bufs=4) as stp,
            tc.tile_pool(name="zi", bufs=4) as zip_,
            tc.tile_pool(name="psh", bufs=2, space="PSUM") as psh,
            tc.tile_pool(name="pso", bufs=2 * n_half, space="PSUM") as pso,
            tc.tile_pool(name="psz", bufs=2, space="PSUM") as psz,
        ):
            # ---- constants ----
            xt = const.tile([P, b_local, n], fp32)
            nc.sync.dma_start(out=xt, in_=xt_d[:].rearrange("b f j -> f b j"))
            wc = const.tile([P, HOPS * f], fp32)
            nc.sync.dma_start(out=wc, in_=wc_d[:])
            ones = const.tile([P, 8], fp32)
            nc.vector.memset(ones, 1.0)
            # bias replicated across partitions and the bg batches of a half
            br = const.tile([P, HOPS, bg, f], fp32)
            for k in range(HOPS):
                bsl = bias_d[k * f:(k + 1) * f]
                bcast = bass.AP(
                    tensor=bsl.tensor,
                    offset=bsl.offset,
                    ap=[[0, P], [0, bg], [1, f]],
                )
                nc.sync.dma_start(out=br[:, k], in_=bcast)

            def h_build(k, ps_pool, evict):
                """H_k[j, (b,f)] = x @ W_k, one PSUM tile per (jc, half)."""
                for jc in range(nch):
                    for h in range(n_half):
                        ps = ps_pool.tile([P, hw], fp32, tag="psh")
                        for bi in range(bg):
                            b = h * bg + bi
                            nc.tensor.matmul(
                                ps[:, bi * f:(bi + 1) * f],
                                xt[:, b, jc * P:(jc + 1) * P],
                                wc[:, k * f:(k + 1) * f],
                                start=True,
                                stop=True,
                            )
                        evict(jc, h, ps)

            # ---- hop 0: A = I  =>  out0 = x @ W0 + bias0 ----
            def evict0(jc, h, ps):
                st = stp.tile([P, hw], fp32, tag="stage")
                nc.vector.tensor_tensor(
                    out=st,
                    in0=ps,
                    in1=br[:, 0].rearrange("p a b -> p (a b)"),
                    op=mybir.AluOpType.add,
                )
                dst = out_d[0, h * bg:(h + 1) * bg, jc * P:(jc + 1) * P, :]
                nc.sync.dma_start(
                    out=dst.rearrange("b i f -> i b f"),
                    in_=st.rearrange("p (b f) -> p b f", b=bg),
                )

            h_build(0, psh, evict0)

            # ---- hops 1..3 ----
            for kk in range(khops):
                k = kk + 1
                # load E_k^T, m_k^T  as [128, nch, n] (partition = j % 128)
                et = ptp.tile([P, nch, n], fp32, tag="pt")
                nc.sync.dma_start(
                    out=et, in_=et_d[kk].rearrange("(c p) i -> p c i", p=P)
                )
                mk = mkp.tile([P, nch, n], mybir.dt.uint8, tag="mk")
                nc.sync.dma_start(
                    out=mk, in_=mt_d[kk].rearrange("(c p) i -> p c i", p=P)
                )

                # H_k
                hh = hhp.tile([P, nch, hc], fp32, tag="hh")

                def evicth(jc, h, ps, hh=hh):
                    nc.scalar.copy(out=hh[:, jc, h * hw:(h + 1) * hw], in_=ps)

                h_build(k, psh, evicth)

                # P = exp(E^T) * m^T  (in place on et), chunked by j-chunk
                for c in range(nch):
                    nc.scalar.activation(
                        out=et[:, c, :],
                        in_=et[:, c, :],
                        func=mybir.ActivationFunctionType.Exp,
                    )
                    nc.vector.tensor_tensor(
                        out=et[:, c, :],
                        in0=et[:, c, :],
                        in1=mk[:, c, :],
                        op=mybir.AluOpType.mult,
                    )

                # main: out[i,(b,f)] = sum_j P^T-tile @ H-tile ; Z via ones
                for ib in range(nch):
                    pz = psz.tile([P, 1], fp32, tag="psz")
                    pos = [
                        pso.tile([P, hw], fp32, tag="pso", name=f"pso_{ib}_{h2}")
                        for h2 in range(n_half)
                    ]
                    for jc in range(nch):
                        lhsT = et[:, jc, ib * P:(ib + 1) * P]
                        st_, sp_ = (jc == 0), (jc == nch - 1)
                        for h in range(n_half):
                            nc.tensor.matmul(
                                pos[h],
                                lhsT,
                                hh[:, jc, h * hw:(h + 1) * hw],
                                start=st_,
                                stop=sp_,
                            )
                        nc.tensor.matmul(
                            pz, lhsT, ones[:, 0:1], start=st_, stop=sp_
                        )
                    zinv = zip_.tile([P, 1], fp32, tag="zi")
                    nc.vector.reciprocal(out=zinv, in_=pz)
                    for h in range(n_half):
                        st = stp.tile([P, hw], fp32, tag="stage")
                        nc.vector.scalar_tensor_tensor(
                            out=st,
                            in0=pos[h],
                            scalar=zinv,
                            in1=br[:, k].rearrange("p a b -> p (a b)"),
                            op0=mybir.AluOpType.mult,
                            op1=mybir.AluOpType.add,
                        )
                        dst = out_d[k, h * bg:(h + 1) * bg, ib * P:(ib + 1) * P, :]
                        nc.sync.dma_start(
                            out=dst.rearrange("b i f -> i b f"),
                            in_=st.rearrange("p (b f) -> p b f", b=bg),
                        )
    return nc


_nc_cache = {}


def _get_nc(b_local, n, f):
    key = (b_local, n, f)
    if key not in _nc_cache:
        _nc_cache[key] = build_nc(b_local, n, f)
    return _nc_cache[key]


def _run(x, W, Es, bias, ms, n_cores, trace=False):
    """x:[B,N,F] W:[4,F,F] Es:[E1,E2,E3] ms:[m1,m2,m3] (hop-0 handled as identity)."""
    b, n, f = x.shape
    b_local = b // n_cores
    nc = _get_nc(b_local, n, f)

    et = np.ascontiguousarray(
        np.stack([e.T for e in Es]).astype(np.float32)
    )
    mt = np.ascontiguousarray(
        np.stack([m.T for m in ms]).astype(np.uint8)
    )
    wc = np.ascontiguousarray(
        np.concatenate([W[k] for k in range(HOPS)], axis=1).astype(np.float32)
    )
    bias = np.ascontiguousarray(bias.astype(np.float32))

    in_maps = []
    for c in range(n_cores):
        xs = x[c * b_local:(c + 1) * b_local]          # [b_local, n, f]
        xts = np.ascontiguousarray(xs.transpose(0, 2, 1).astype(np.float32))
        in_maps.append({"xt": xts, "et": et, "mt": mt, "wc": wc, "bias": bias})

    last_run_info["nc"] = nc
    last_run_info["in_maps"] = in_maps
    res = run_bass_kernel_spmd(
        nc, in_maps, core_ids=list(range(n_cores)), trace=trace
    )
    last_run_info["exec_time_ns"] = res.exec_time_ns
    last_run_info["trace"] = res.instructions_and_trace

    out = np.empty((b, n, HOPS * f), dtype=np.float32)
    for c in range(n_cores):
        od = res.results[c]["out"]                     # [HOPS, b_local, n, f]
        for k in range(HOPS):
            out[c * b_local:(c + 1) * b_local, :, k * f:(k + 1) * f] = od[k]
    return out


def build_null_nc(b_local: int, n: int, f: int = 128):
    """Same external tensors as build_nc but ~no device work — used to
    subtract host/transfer/dispatch overhead when estimating HW exec time."""
    P = 128
    khops = HOPS - 1
    nc = bass.Bass()
    fp32 = mybir.dt.float32
    nc.dram_tensor("xt", [b_local, f, n], fp32, kind="ExternalInput")
    nc.dram_tensor("et", [khops, n, n], fp32, kind="ExternalInput")
    nc.dram_tensor("mt", [khops, n, n], mybir.dt.uint8, kind="ExternalInput")
    wc_d = nc.dram_tensor("wc", [f, HOPS * f], fp32, kind="ExternalInput")
    nc.dram_tensor("bias", [HOPS * f], fp32, kind="ExternalInput")
    out_d = nc.dram_tensor("out", [HOPS, b_local, n, f], fp32, kind="ExternalOutput")
    with tile.TileContext(nc) as tc:
        with tc.tile_pool(name="p", bufs=1) as pool:
            t = pool.tile([P, 8], fp32)
            nc.sync.dma_start(out=t, in_=wc_d[:, 0:8])
            nc.sync.dma_start(out=out_d[0, 0, 0:P, 0:8], in_=t)
    return nc


def time_exec(iters=3):
    """Re-execute the last-run kernel and a null kernel; return
    (min_real_s, min_null_s). Uses identical input tensors so transfer and
    dispatch overhead cancels in the difference."""
    import time as _t

    nc = last_run_info["nc"]
    in_maps = last_run_info["in_maps"]
    n_cores = len(in_maps)
    reals, nulls = [], []
    for _ in range(iters):
        t0 = _t.time()
        run_bass_kernel_spmd(nc, in_maps, core_ids=list(range(n_cores)))
        reals.append(_t.time() - t0)
    b_local, f, n = in_maps[0]["xt"].shape
    nnc = build_null_nc(b_local, n, f)
    for _ in range(iters):
        t0 = _t.time()
        run_bass_kernel_spmd(nnc, in_maps, core_ids=list(range(n_cores)))
        nulls.append(_t.time() - t0)
    return min(reals), min(nulls), reals, nulls


def kernel(**inputs) -> np.ndarray:
    x = np.asarray(inputs["x"], dtype=np.float32)
    W = np.asarray(inputs["W"], dtype=np.float32)
    Es = [np.asarray(inputs[f"E{i}"], dtype=np.float32) for i in range(4)]
    bias = np.asarray(inputs["bias"], dtype=np.float32)
    ms = [np.asarray(inputs[f"m{i}"]).astype(bool) for i in range(4)]

    trace = bool(int(os.environ.get("HGRAPH_TRACE", "0")))
    out = _run(x, W, Es[1:], bias, ms[1:], N_CORES, trace=trace)

    f = W.shape[2]
    n = x.shape[1]
    # Safety net 1: hop 0 assumes m0 == I (structurally true for this module).
    if not np.array_equal(ms[0], np.eye(n, dtype=bool)):
        s0 = np.where(ms[0], Es[0], NEG)
        s0 = s0 - s0.max(axis=1, keepdims=True)
        p0 = np.exp(s0)
        a0 = p0 / p0.sum(axis=1, keepdims=True)
        h0 = np.einsum("bnf,fo->bno", x, W[0])
        out[:, :, 0:f] = np.einsum("ij,bjo->bio", a0, h0) + bias[None, None, :f]
    # Safety net 2: all-masked rows (softmax -> uniform; device would give NaN).
    for k in range(1, 4):
        empty = ~ms[k].any(axis=1)
        if empty.any():
            hk = np.einsum("bnf,fo->bno", x, W[k])
            unif = hk.mean(axis=1)  # [B, f]
            idx = np.where(empty)[0]
            out[:, idx, k * f:(k + 1) * f] = unif[:, None, :] + bias[None, None, k * f:(k + 1) * f]
    return out
